# revision 1
# baseline (speedup 1.0000x reference)
"""Bass/Trainium2 kernel for the GaussianRecu (Kalman-style linear scan) model.

Reference recursion (C = I, dt = 0.01), per batch b, scanned over t:
    out_t   = dt * x_t                      (emitted before update)
    x_{t+1} = x_t + dt*(A - cov_t) x_t + cov_t dy_t
    cov_{t+1} = cov_t A + A cov_t

The cov recursion is linear with spectral radius 2*rho(A); for contracting A
it underflows to EXACT fp32 zero after a few dozen steps.  Once cov == 0
exactly, the remaining recursion is exactly x <- x + dt*(A x), i.e.
    out[b, t, :] = W_t @ x*(b),   W_t = dt * G^(t-t0),  G = I + dt*A.

So: simulate the first t0 steps on host in exact fp32 (tiny), precompute the
2x2 power coefficients W_t in fp64 (tiny), and let the device generate the
full (B, T, 2) output as a rank-2 broadcast:
    out[b, t, i] = W0[t, i] * x*(b, 0) + W1[t, i] * x*(b, 1)
which is memory-roofline work: 8 MB of output writes per core.

Sharding: pure data parallel, batch 128 -> 16 rows per core on 8 cores.
"""

import numpy as np

B, T = 128, 65536
DT32 = np.float32(0.01)
N_CORES = 8
BPC = B // N_CORES  # 16 batch rows per core
P = 128             # SBUF partitions
ROW = T * 2         # flattened (t, i) length per batch row
F = ROW // P        # free-dim columns per partition (1024)

TRACE = False          # test harness may set True to collect a HW profile
LAST_RESULTS = None    # BassKernelResults of the most recent device run

DELTA = 128            # t-shift of the second basis view (shifted-basis mode)
_PROGRAMS = {}         # cached Bass programs by variant


def _build_program(shifted):
    """Device program.

    shifted=True: ONE overlapped coefficient plane `r` (P, F + 2*DELTA);
      the two broadcast operands are column views r[:, 0:F] (basis row for
      seed u) and r[:, 2D:2D+F] (the same plane shifted by DELTA t-steps,
      i.e. the basis row for seed G^DELTA u).  Loads 640KB instead of 1MB.
    shifted=False: legacy two full planes w0/w1 (fallback for degenerate A).

    In-load issue assignment: each dma_start blocks its issuing engine
    ~0.6us and gpsimd/SWDGE adds a ~4us drain, so loads go on the two
    HWDGE engines (scalar + sync), partition-half split for parallel
    queues with >=4KB-per-partition descriptors.  The load phase is
    HBM-read-bound; reordering/finer splits measured neutral or worse.
    """
    import concourse.bacc as bacc
    import concourse.tile as tile
    from concourse import mybir

    f32 = mybir.dt.float32
    nc = bacc.Bacc(
        "TRN2", target_bir_lowering=False, debug=False, num_devices=N_CORES
    )
    if shifted:
        r = nc.declare_dram_parameter(
            "r", [P, F + 2 * DELTA], f32, isOutput=False
        )
    else:
        w0 = nc.declare_dram_parameter("w0", [P, F], f32, isOutput=False)
        w1 = nc.declare_dram_parameter("w1", [P, F], f32, isOutput=False)
    xs = nc.declare_dram_parameter("xs", [P, 2 * BPC], f32, isOutput=False)
    out = nc.declare_dram_parameter("out", [BPC, P, F], f32, isOutput=True)

    with tile.TileContext(nc) as tc:
        with (
            tc.tile_pool(name="consts", bufs=1) as consts,
            tc.tile_pool(name="ot", bufs=8) as otp,
        ):
            xst = consts.tile([P, 2 * BPC], f32)
            PH = P // 2
            if shifted:
                # R halves issue FIRST on both queues (their transfers gate
                # the first compute); the tiny xs load rides behind.
                rt = consts.tile([P, F + 2 * DELTA], f32)
                nc.scalar.dma_start(out=rt[0:PH, :], in_=r[0:PH, :])
                nc.sync.dma_start(out=rt[PH:P, :], in_=r[PH:P, :])
                nc.scalar.dma_start(out=xst[:], in_=xs[:])
                v0 = rt[:, 0:F]
                v1 = rt[:, 2 * DELTA : 2 * DELTA + F]
            else:
                w0t = consts.tile([P, F], f32)
                w1t = consts.tile([P, F], f32)
                for c in range(2):
                    sl = slice(c * PH, (c + 1) * PH)
                    nc.scalar.dma_start(out=w0t[sl, :], in_=w0[sl, :])
                    nc.sync.dma_start(out=w1t[sl, :], in_=w1[sl, :])
                nc.scalar.dma_start(out=xst[:], in_=xs[:])
                v0 = w0t[:]
                v1 = w1t[:]

            for b in range(BPC):
                o = otp.tile([P, F], f32)
                s0 = xst[:, 2 * b : 2 * b + 1]
                s1 = xst[:, 2 * b + 1 : 2 * b + 2]
                # o = V0 * alpha_b.  Row 0 multiplies on DVE (2x-mode
                # tensor_scalar, no cross-engine handoff, and it keeps the
                # ACT-table load off the critical path); later rows on ACT
                # so the two engines pipeline.
                if b == 0:
                    nc.vector.tensor_scalar_mul(o[:], v0, s0)
                else:
                    nc.scalar.mul(o[:], v0, mul=s0)
                # o = V1 * beta_b + o    (DVE fused multiply-add)
                nc.vector.scalar_tensor_tensor(
                    out=o[:],
                    in0=v1,
                    scalar=s1,
                    in1=o[:],
                    op0=mybir.AluOpType.mult,
                    op1=mybir.AluOpType.add,
                )
                nc.sync.dma_start(out=out[b], in_=o[:])
    nc.compile()
    return nc


def _early_phase(dy, x0, cov0, A32):
    """Exact fp32 replica of the reference scan until cov == 0 exactly.

    Returns (early_out (B, t0, 2), xstar (B, 2), t0)."""
    x = x0.astype(np.float32).copy()
    cov = cov0.astype(np.float32).copy()
    rows = []
    t = 0
    while t < T and not np.all(cov == 0):
        rows.append(x * DT32)
        K = A32[None, :, :] - cov
        dx = np.einsum("bij,bj->bi", K, x) * DT32 + np.einsum(
            "bij,bj->bi", cov, dy[:, t, :]
        )
        cov = np.einsum("bij,jk->bik", cov, A32) + np.einsum(
            "ij,bjk->bik", A32, cov
        )
        x = x + dx
        t += 1
    early = (
        np.stack(rows, axis=1) if rows else np.zeros((B, 0, 2), np.float32)
    )
    return early.astype(np.float32), x, t


def _powers(A, n):
    """G^k for k in [0, n), fp64 block products; G = I + dt*A."""
    dtv = float(DT32)
    G = np.eye(2, dtype=np.float64) + dtv * A.astype(np.float64)
    S = 1024
    Ps = np.empty((S, 2, 2), np.float64)
    cur = np.eye(2, dtype=np.float64)
    for s in range(S):
        Ps[s] = cur
        cur = cur @ G
    GS = cur  # G^S
    M = (n + S - 1) // S
    Cs = np.empty((M, 2, 2), np.float64)
    cur = np.eye(2, dtype=np.float64)
    for m in range(M):
        Cs[m] = cur
        cur = cur @ GS
    # G^(m*S + s) = G^(m*S) @ G^s
    return np.einsum("mij,sjk->msik", Cs, Ps).reshape(M * S, 2, 2)[:n]


def kernel(dy, x0, cov0, A):
    global LAST_RESULTS
    from concourse.bass_utils import run_bass_kernel_spmd

    dy = np.ascontiguousarray(np.asarray(dy, dtype=np.float32))
    x0 = np.asarray(x0, dtype=np.float32)
    cov0 = np.asarray(cov0, dtype=np.float32)
    A32 = np.asarray(A, dtype=np.float32)
    assert dy.shape == (B, T, 2) and x0.shape == (B, 2)

    early, xstar, t0 = _early_phase(dy, x0, cov0, A32)
    K = T - t0
    dtv = float(DT32)

    # Shifted-basis mode: one plane R[t] = dt*G^(t-t0) u plus its DELTA-
    # shifted view spans the same space as {W0, W1} when [u, G^D u] is
    # well-conditioned; coefficients solve [u, G^D u] @ (a, b) = x*.
    shifted = False
    if K > 0:
        Gpow = _powers(A32, K + DELTA)
        GD = Gpow[DELTA]
        cands = [(1.0, 0.0), (0.0, 1.0), (0.7071, 0.7071), (0.7071, -0.7071)]
        best_u, best_q = None, 0.0
        for cu in cands:
            u = np.array(cu, np.float64)
            v = GD @ u
            q = abs(u[0] * v[1] - u[1] * v[0]) / (
                np.linalg.norm(u) * np.linalg.norm(v) + 1e-300
            )
            if q > best_q:
                best_u, best_q = u, q
        shifted = best_q > 1e-4

    if shifted:
        Rvals = (Gpow @ best_u) * dtv  # (K+DELTA, 2) = (W_t u)_i
        Rflat = np.zeros((2 * (T + DELTA),), np.float64)
        Rflat[2 * t0 :] = Rvals.reshape(-1)
        R32 = Rflat.astype(np.float32)
        idx = np.arange(P)[:, None] * F + np.arange(F + 2 * DELTA)[None, :]
        w_inputs = {"r": np.ascontiguousarray(R32[idx])}
        M2 = np.column_stack([best_u, GD @ best_u])
        coef = np.linalg.solve(M2, xstar.T.astype(np.float64)).T.astype(
            np.float32
        )  # (B, 2) = (alpha, beta)
    else:
        Wflat0 = np.zeros((T, 2), np.float64)
        Wflat1 = np.zeros((T, 2), np.float64)
        if K > 0:
            Wfull = Gpow[:K] * dtv
            Wflat0[t0:, :] = Wfull[:, :, 0]
            Wflat1[t0:, :] = Wfull[:, :, 1]
        w_inputs = {
            "w0": Wflat0.astype(np.float32).reshape(P, F),
            "w1": Wflat1.astype(np.float32).reshape(P, F),
        }
        coef = xstar

    if shifted not in _PROGRAMS:
        _PROGRAMS[shifted] = _build_program(shifted)
    nc = _PROGRAMS[shifted]

    in_maps = []
    for r in range(N_CORES):
        xs_core = np.tile(
            coef[r * BPC : (r + 1) * BPC].reshape(1, 2 * BPC), (P, 1)
        ).astype(np.float32)
        in_maps.append({**w_inputs, "xs": np.ascontiguousarray(xs_core)})

    res = run_bass_kernel_spmd(nc, in_maps, list(range(N_CORES)), trace=TRACE)
    LAST_RESULTS = res

    full = np.concatenate(
        [res.results[r]["out"].reshape(BPC, T, 2) for r in range(N_CORES)],
        axis=0,
    )
    if t0 > 0:
        full[:, :t0, :] = early
    return np.ascontiguousarray(full.astype(np.float32, copy=False))

